# revision 1
# baseline (speedup 1.0000x reference)
"""DNRI MLP decoder kernel for 8 Trainium2 NeuronCores.

Strategy (data-parallel on batch, 8 batches/core):
  - Full 64x64 sender/receiver grid (4096 >= E=4032 edges); edge weights are
    scattered into a dense [recv, send] grid host-side (zero diagonal), which
    makes the per-edge gather/scatter fully affine on device.
  - Layout A on device: channels on partitions, edge-items on the free dim.
    Items ordered recv-major: item = r*64 + s.
  - fc1 bias folded into the matmul via a ones-row (K=65).
  - Per-edge weighting: weight rows replicated across partitions by a
    partition-step-0 broadcast DMA (idle DMA engines), then a custom fused
    DVE op relu(fc2_psum + bias) * weight; type-sum via DVE adds.
  - Scatter-add over senders is folded into the output head by linearity:
    O1m @ (sum_s msgs) = sum_s (O1m @ msg_slice_s) as 64 PSUM-accumulating
    N=64 matmuls per batch on the (underutilized) tensor engine.
  - Output heads run as a second phase so their PSUM use never contends
    with the main loop; mu matmul emits node-major output directly.
"""

import sys

import numpy as np

if "/opt/trn_rl_repo" not in sys.path:
    sys.path.insert(0, "/opt/trn_rl_repo")

import ml_dtypes  # noqa: E402

import concourse.bass as bass  # noqa: E402
import concourse.bacc as bacc  # noqa: E402
import concourse.mybir as mybir  # noqa: E402
from concourse import tile  # noqa: E402

NUM_VARS = 64
HID = 128
IN_F = 32
BATCH = 64
N_CORES = 8
BC = BATCH // N_CORES  # batches per core
NT = 3  # edge types used (SKIP_FIRST drops type 0)
RBLK = 8  # recv rows per tile
T = RBLK * NUM_VARS  # 512 items per tile
NTILES = NUM_VARS // RBLK  # 8 tiles per batch

F32 = mybir.dt.float32
BF16 = mybir.dt.bfloat16

COMPUTE_DT = BF16  # compute dtype for activations/weights
NP_CDT = ml_dtypes.bfloat16 if COMPUTE_DT == BF16 else np.float32

_CACHED = {}


def _register_fused_op():
    """Register a custom DVE op: out = relu(in0 + s0) * in1
    (fc2 bias-add + relu + per-edge weight multiply in one instruction)."""
    import numpy as _np

    from concourse import dve_ops as _do
    from concourse.dve_spec import Spec, Src0, Src1, C0, relu
    from concourse.dve_table_gen import dve_ver_for
    from concourse.dve_uop import DveOpSpec
    from concourse.dve_ops import DveOp, has_src1
    from concourse.dve_spec import lower as _lower

    name = "RELU_BIAS_MUL_K77"
    if any(op.name == name for op in _do.OPS):
        return next(op for op in _do.OPS if op.name == name)

    spec = Spec(
        body=relu(Src0 + C0) * Src1,
        reference=lambda in0, in1, s0, s1, imm2: (
            _np.maximum(in0.astype(_np.float32) + s0, 0) * in1
        ),
    )
    op = DveOp(name, spec, subdim=False, uops_sha={})
    opcode = _do._CUSTOM_DVE_ROW_BASE + len(_do.OPS)
    _do.OPS.append(op)
    _do.CUSTOM_DVE_SPECS[name] = spec
    _do._SUB_OPCODE_FOR_NAME[name] = opcode
    # pin the uops sha (computed locally; compile() checks it)
    for ver in ("v3", "v4"):
        try:
            s = DveOpSpec(
                name=name, opcode=opcode,
                uops=_lower(spec, ver=ver), rd1_en=has_src1(spec),
            )
            op.uops_sha[ver] = s.sha(ver)
        except Exception:
            pass
    return op


def build_kernel(cdt=COMPUTE_DT):
    fused_op = _register_fused_op()
    nc = bacc.Bacc("TRN2", target_bir_lowering=False)

    xT_d = nc.dram_tensor("xT", [BC, IN_F + 1, NUM_VARS], cdt, kind="ExternalInput")
    xrec_d = nc.dram_tensor(
        "xrec", [BC, IN_F + 1, NUM_VARS * NUM_VARS], cdt, kind="ExternalInput"
    )
    xres_d = nc.dram_tensor("x_res", [BC, NUM_VARS, IN_F], F32, kind="ExternalInput")
    wg_d = nc.dram_tensor("wg", [BC, NT, NUM_VARS * NUM_VARS], cdt, kind="ExternalInput")
    W1_d = nc.dram_tensor("W1l", [NT, 2 * IN_F + 1, HID], cdt, kind="ExternalInput")
    W2T_d = nc.dram_tensor("W2T", [NT, HID, HID], cdt, kind="ExternalInput")
    b2_d = nc.dram_tensor("b2", [NT, HID, 1], F32, kind="ExternalInput")
    O1x_d = nc.dram_tensor("O1x", [IN_F + 1, HID], cdt, kind="ExternalInput")
    O1m_d = nc.dram_tensor("O1m", [HID, HID], cdt, kind="ExternalInput")
    O2T_d = nc.dram_tensor("O2T", [HID, HID], cdt, kind="ExternalInput")
    bo2_d = nc.dram_tensor("bo2", [HID, 1], F32, kind="ExternalInput")
    muT_d = nc.dram_tensor("muT", [HID, IN_F], cdt, kind="ExternalInput")
    mub_d = nc.dram_tensor("mub", [NUM_VARS, IN_F], F32, kind="ExternalInput")
    out_d = nc.dram_tensor("out", [BC, NUM_VARS, IN_F], F32, kind="ExternalOutput")

    AL = mybir.AluOpType

    with tile.TileContext(nc) as tc:
        with (
            tc.tile_pool(name="const", bufs=1) as cpool,
            tc.tile_pool(name="perb", bufs=8) as bpool,
            tc.tile_pool(name="keep", bufs=BC) as kpool,
            tc.tile_pool(name="work", bufs=2) as wpool,
            tc.tile_pool(name="acts", bufs=6) as apool,
            tc.tile_pool(name="accs", bufs=6) as accpool,
            tc.tile_pool(name="ps", bufs=2, space="PSUM") as pspool,
        ):
            # ---- constants ----
            W1_sb = cpool.tile([2 * IN_F + 1, NT * HID], cdt, tag="W1")
            W2T_sb = cpool.tile([HID, NT * HID], cdt, tag="W2T")
            b2_sb = cpool.tile([HID, NT], F32, tag="b2")
            O1x_sb = cpool.tile([IN_F + 1, HID], cdt, tag="O1x")
            O1m_sb = cpool.tile([HID, HID], cdt, tag="O1m")
            O2T_sb = cpool.tile([HID, HID], cdt, tag="O2T")
            bo2_sb = cpool.tile([HID, 1], F32, tag="bo2")
            muT_sb = cpool.tile([HID, IN_F], cdt, tag="muT")
            mub_sb = cpool.tile([NUM_VARS, IN_F], F32, tag="mub")

            for i in range(NT):
                nc.sync.dma_start(W1_sb[:, i * HID:(i + 1) * HID], W1_d[i])
                nc.sync.dma_start(W2T_sb[:, i * HID:(i + 1) * HID], W2T_d[i])
                nc.sync.dma_start(b2_sb[:, i:i + 1], b2_d[i])
            nc.sync.dma_start(O1x_sb[:], O1x_d[:])
            nc.sync.dma_start(O1m_sb[:], O1m_d[:])
            nc.sync.dma_start(O2T_sb[:], O2T_d[:])
            nc.sync.dma_start(bo2_sb[:], bo2_d[:])
            nc.sync.dma_start(muT_sb[:], muT_d[:])
            nc.sync.dma_start(mub_sb[:], mub_d[:])

            GR = NUM_VARS * NUM_VARS  # 4096 items per batch
            TB = 1024 if cdt == BF16 else 512  # tile (bf16 rhs max 1024)
            RB2 = TB // NUM_VARS  # recv rows per tile
            wdma = [nc.gpsimd, nc.scalar, nc.sync]
            xTs, xress, accbs = [], [], []
            for b in range(BC):
                xT_sb = kpool.tile([IN_F + 1, NUM_VARS], cdt, tag="xT")
                xres_sb = kpool.tile([NUM_VARS, IN_F], F32, tag="xres")
                accb = kpool.tile([HID, GR], cdt, tag="accb")
                xTs.append(xT_sb)
                xress.append(xres_sb)
                accbs.append(accb)

                nc.sync.dma_start(xT_sb[:], xT_d[b])
                nc.gpsimd.dma_start(xres_sb[:], xres_d[b])

                # whole-batch pre tile: [recv(32); ones(1); send(32)] x 4096
                pre = wpool.tile([2 * IN_F + 1, GR], cdt, tag="pre")
                nc.scalar.dma_start(pre[0:IN_F + 1, :], xrec_d[b])
                nc.sync.dma_start(
                    pre[IN_F + 1:2 * IN_F + 1, :]
                    .rearrange("p (a b) -> p a b", a=NUM_VARS),
                    xT_d[b, 0:IN_F, :]
                    .unsqueeze(1)
                    .to_broadcast([IN_F, NUM_VARS, NUM_VARS]),
                )
                # edge-weight rows broadcast across all partitions (DMA bcast)
                wbs = []
                for i in range(NT):
                    wb = wpool.tile([HID, GR], cdt, tag=f"wb{i}")
                    wdma[i].dma_start(
                        wb[:], wg_d[b, i].unsqueeze(0).to_broadcast([HID, GR])
                    )
                    wbs.append(wb)

                for jb in range(GR // TB):
                    c0 = jb * TB
                    r0 = c0 // NUM_VARS
                    acc = accb[:, c0:c0 + TB]
                    # ACT-path type (2) first: its multiply writes acc
                    # directly, so the closing DVE adds never wait on a
                    # late ACT relu
                    for idx, i in enumerate((2, 0, 1)):
                        ps1 = pspool.tile([HID, TB], F32, tag="ps1")
                        for h in range(TB // 512):
                            nc.tensor.matmul(
                                ps1[:, h * 512:(h + 1) * 512],
                                W1_sb[:, i * HID:(i + 1) * HID],
                                pre[:, c0 + h * 512:c0 + (h + 1) * 512],
                            )
                        m1 = apool.tile([HID, TB], cdt, tag="m1")
                        nc.scalar.activation(
                            m1[:], ps1[:], mybir.ActivationFunctionType.Relu
                        )
                        ps2 = pspool.tile([HID, TB], F32, tag="ps2")
                        for h in range(TB // 512):
                            nc.tensor.matmul(
                                ps2[:, h * 512:(h + 1) * 512],
                                W2T_sb[:, i * HID:(i + 1) * HID],
                                m1[:, h * 512:(h + 1) * 512],
                            )
                        wslice = wbs[i][:, c0:c0 + TB]
                        # fused relu+bias+weight on DVE (custom op) for some
                        # types; ACT relu+bias + DVE multiply for the rest
                        # (alternating per tile to balance ACT vs DVE load)
                        if i < 2:
                            dst = accpool.tile([HID, TB], cdt, tag="tmp")
                            nc.vector._custom_dve(
                                fused_op, out=dst[:], in0=ps2[:],
                                in1=wslice, s0=b2_sb[:, i:i + 1],
                            )
                            nc.vector.tensor_tensor(
                                acc[:], acc[:], dst[:], AL.add
                            )
                        else:
                            m2 = apool.tile([HID, TB], cdt, tag="m2")
                            nc.scalar.activation(
                                m2[:], ps2[:],
                                mybir.ActivationFunctionType.Relu,
                                bias=b2_sb[:, i:i + 1],
                            )
                            nc.vector.tensor_tensor(
                                acc[:], m2[:], wslice, AL.mult
                            )

            # ---- output heads (second phase: no PSUM contention with the
            # main loop, and the 8 small per-batch chains pipeline together)
            for b in range(BC):
                xT_sb, xres_sb = xTs[b], xress[b]
                # scatter-add over senders folded into out_fc1 by linearity:
                # O1m @ (sum_s msgs) = sum_s (O1m @ msg_slice_s), accumulated
                # in PSUM over 64 strided N=64 matmuls (PE has headroom)
                av = accbs[b][:].rearrange("p (r s) -> p s r", r=NUM_VARS)
                pso1 = pspool.tile([HID, NUM_VARS], F32, tag="ps2")
                nc.tensor.matmul(pso1[:], O1x_sb[:], xT_sb[:], start=True, stop=False)
                for s in range(NUM_VARS):
                    nc.tensor.matmul(
                        pso1[:], O1m_sb[:], av[:, s, :],
                        start=False, stop=(s == NUM_VARS - 1),
                    )
                pred1 = bpool.tile([HID, NUM_VARS], cdt, tag="pred1")
                nc.scalar.activation(
                    pred1[:], pso1[:], mybir.ActivationFunctionType.Relu
                )
                pso2 = pspool.tile([HID, NUM_VARS], F32, tag="ps1")
                nc.tensor.matmul(pso2[:], O2T_sb[:], pred1[:])
                pred2 = bpool.tile([HID, NUM_VARS], cdt, tag="pred2")
                nc.scalar.activation(
                    pred2[:],
                    pso2[:],
                    mybir.ActivationFunctionType.Relu,
                    bias=bo2_sb[:],
                )
                # mu matmul with pred2 as stationary -> node-major [64, 32]
                psmu = pspool.tile([NUM_VARS, IN_F], F32, tag="ps2")
                nc.tensor.matmul(psmu[:], pred2[:], muT_sb[:])
                out_sb = bpool.tile([NUM_VARS, IN_F], F32, tag="outsb")
                nc.vector.tensor_tensor(out_sb[:], psmu[:], xres_sb[:], AL.add)
                nc.vector.tensor_tensor(out_sb[:], out_sb[:], mub_sb[:], AL.add)
                nc.gpsimd.dma_start(out_d[b], out_sb[:])

    nc.finalize()
    return nc


def prep_inputs(inputs, edges, msg_fc1_w, msg_fc1_b, msg_fc2_w, msg_fc2_b,
                out_fc1_w, out_fc1_b, out_fc2_w, out_fc2_b,
                mu_w, mu_b, logstd_w, logstd_b, send_edges, recv_edges):
    """Build the per-core input maps (host-side shard + repack)."""
    inputs = np.asarray(inputs, np.float32)
    edges = np.asarray(edges, np.float32)
    send = np.asarray(send_edges, np.int64)
    recv = np.asarray(recv_edges, np.int64)

    B = inputs.shape[0]
    # dense [recv, send] weight grid per (batch, type); np.add.at handles
    # duplicate (send, recv) pairs exactly
    wg = np.zeros((B, NT, NUM_VARS * NUM_VARS), np.float32)
    idx = recv * NUM_VARS + send
    ed = edges[:, :, 1:1 + NT].transpose(0, 2, 1).reshape(B * NT, -1)
    wgf = wg.reshape(B * NT, -1)
    np.add.at(wgf, (slice(None), idx), ed)

    ones_b = np.ones((B, 1, NUM_VARS), np.float32)
    xT = np.concatenate([inputs.transpose(0, 2, 1), ones_b], axis=1)  # [B,33,64]

    # rows: [0:32]=recv-part, [32]=bias, [33:65]=send-part (matches pre layout)
    W1l = np.concatenate(
        [
            msg_fc1_w[1:, :, :IN_F].transpose(0, 2, 1),
            msg_fc1_b[1:, None, :],
            msg_fc1_w[1:, :, IN_F:].transpose(0, 2, 1),
        ],
        axis=1,
    )  # [3,65,128]
    W2T = msg_fc2_w[1:].transpose(0, 2, 1)  # [3,128,128]
    b2 = np.ascontiguousarray(msg_fc2_b[1:, :, None], dtype=np.float32)
    O1x = np.concatenate([out_fc1_w[:, :IN_F].T, out_fc1_b[None, :]], axis=0)
    O1m = np.ascontiguousarray(out_fc1_w[:, IN_F:].T)
    O2T = np.ascontiguousarray(out_fc2_w.T)
    bo2 = np.ascontiguousarray(out_fc2_b[:, None], dtype=np.float32)
    muT = np.ascontiguousarray(mu_w.T)
    mub = np.broadcast_to(mu_b[None, :], (NUM_VARS, IN_F)).copy()

    def c(a):
        return np.ascontiguousarray(a, dtype=NP_CDT)

    shared = {
        "W1l": c(W1l), "W2T": c(W2T), "b2": b2.astype(np.float32),
        "O1x": c(O1x), "O1m": c(O1m), "O2T": c(O2T),
        "bo2": bo2, "muT": c(muT), "mub": mub.astype(np.float32),
    }
    in_maps = []
    for core in range(N_CORES):
        lo, hi = core * BC, (core + 1) * BC
        m = dict(shared)
        m["xT"] = c(xT[lo:hi])
        xr = np.repeat(
            inputs[lo:hi].transpose(0, 2, 1), NUM_VARS, axis=2
        ).reshape(BC, IN_F, -1)
        m["xrec"] = c(
            np.concatenate(
                [xr, np.ones((BC, 1, xr.shape[2]), np.float32)], axis=1
            )
        )
        m["x_res"] = np.ascontiguousarray(inputs[lo:hi], np.float32)
        m["wg"] = c(wg[lo:hi].reshape(BC, NT, -1))
        in_maps.append(m)
    return in_maps


def kernel(**inputs):
    from concourse.bass_utils import run_bass_kernel_spmd

    if "nc" not in _CACHED:
        _CACHED["nc"] = build_kernel()
    nc = _CACHED["nc"]
    in_maps = prep_inputs(**inputs)
    res = run_bass_kernel_spmd(nc, in_maps, core_ids=list(range(N_CORES)))
    out = np.concatenate([r["out"] for r in res.results], axis=0)
    return out.astype(np.float32)

